# revision 1
# baseline (speedup 1.0000x reference)
"""Trainium2 Bass kernel for the MCAT gated-attention MIL pooling model.

Math (from the reference, after dead-code elimination):
  The per-instance "cross attention" softmax is over a length-1 axis, so
  attn_w == 1 exactly and fused = v = relu(x_path @ wsi_w + wsi_b) @ wv_w + wv_b.
  The whole x_cell / wq / wk branch is dead.

  Remaining work (N = 50000 rows):
      h   = relu(x @ W1 + b1)          (N, 256)   <- x (N, 1024)
      f   = h @ Wv + bv                (N, 256)
      a   = tanh(f @ Wa + ba)
      b   = sigmoid(f @ Wb + bb)
      A   = (a*b) @ ac_w + ac_b        (N, 1)
      pooled = softmax(A^T) @ f        (1, 256)
      risk = relu(pooled @ c1 + b) @ c2 + b2     (1, 4)

  |A| < 0.1 for this data, so softmax is computed unnormalized:
  S = sum_n exp(A_n) f_n, Z = sum_n exp(A_n), pooled = S/Z.

Sharding: rows split across 8 cores (6250 each); cores return per-block
partial sums S (128,2,NB) and Z (1,NB); host reduces + tiny classifier.

Performance notes:
  * All matmuls run in float32r (e8m11, 1 cycle/row on the PE vs 4 for fp32).
    Inputs are pre-rounded host-side (RNE to 11 mantissa bits) so the BIR
    verifier's "rounded to FP32r" rule is satisfied with plain HWDGE copies.
  * Accuracy is recovered where it matters: Wv is shipped as a
    round(W) + round(W - round(W)) pair and both halves accumulate into the
    same PSUM tile (x/h rounding is incoherent across rows and averages out
    in the pooling sum; the gating path's softmax-weight perturbations wash
    out in S/Z).  Measured end-to-end rel err: 1.3e-5.  Adding "w1" to SPLIT
    gives 3.6e-6 at +35% runtime (188us vs 140us); fp32 everywhere gives
    1.1e-7 at 375us.
  * sigmoid(y) is computed as 0.5*(1 + tanh(y/2)) so every ACT function used
    (tanh/exp/relu-free) lives in the one "exp_and_others" table set - no
    ~2.7us ACT_TABLE_LOAD switches per block.  The 0.5 factor is folded into
    ac_w on the host; bias/relu epilogues run on the DVE.
  * exp's per-block Z sum uses the ACT accumulator; the softmax-weight
    broadcast to 128 partitions runs on the idle GpSimd engine.
"""

import sys
from contextlib import ExitStack

import numpy as np

try:
    import concourse  # noqa: F401
except ImportError:  # pragma: no cover - fresh grading env
    sys.path.insert(0, "/opt/trn_rl_repo")

import concourse.bass as bass
import concourse.tile as tile
from concourse import bacc, mybir
from concourse.bass_utils import run_bass_kernel_spmd

N_CORES = 8
N = 50000
NPC = N // N_CORES  # 6250 rows per core
D_IN = 1024
D_HID = 256
NB = 512  # rows per block (one PSUM bank of fp32)
SPLIT = ("wv",)  # weights shipped as hi+lo f32r pairs

F32 = mybir.dt.float32
F32R = mybir.dt.float32r
AF = mybir.ActivationFunctionType
ALU = mybir.AluOpType


def rne11(a: np.ndarray) -> np.ndarray:
    """Round fp32 to f32r (RNE to 11 explicit mantissa bits) host-side."""
    b = np.ascontiguousarray(a, np.float32).view(np.uint32)
    out = ((b + np.uint32(1 << 11)) & np.uint32(0xFFFFF000)).view(np.float32)
    return np.ascontiguousarray(out)


def _build_tile_kernel(ctx: ExitStack, tc: tile.TileContext, t, npc: int, nblocks: int, split):
    nc = tc.nc

    singles = ctx.enter_context(tc.tile_pool(name="singles", bufs=1))
    xpool = ctx.enter_context(tc.tile_pool(name="xp", bufs=5))
    actp = ctx.enter_context(tc.tile_pool(name="actp", bufs=3))
    psum = ctx.enter_context(tc.tile_pool(name="psum", bufs=2, space=bass.MemorySpace.PSUM))

    # Block-0 x DMA first in program order: it is on the PE's critical path
    # (weights ride a separate HWDGE ring and overlap it).
    x_tiles0 = xpool.tile([128, 8, NB], F32R, tag="x")
    nc.sync.dma_start(
        out=x_tiles0,
        in_=t["xt"][:, 0 : 8 * NB].rearrange("p (c j) -> p c j", j=NB),
    )

    # ---- persistent weights / biases in SBUF --------------------------------
    def wtile(name, shape, pattern):
        sb = singles.tile(shape, F32R, name=name)
        nc.scalar.dma_start(out=sb, in_=t[name].rearrange(pattern, p=128, j=128))
        return sb

    w1_parts = [wtile("w1h", [128, 8, 2, 128], "(c p) (m j) -> p c m j")]
    if "w1" in split:
        w1_parts.append(wtile("w1l", [128, 8, 2, 128], "(c p) (m j) -> p c m j"))
    wv_parts = [wtile("wvh", [128, 2, 2, 128], "(k p) (m j) -> p k m j")]
    if "wv" in split:
        wv_parts.append(wtile("wvl", [128, 2, 2, 128], "(k p) (m j) -> p k m j"))
    wa_sb = wtile("wah", [128, 2, 2, 128], "(k p) (m j) -> p k m j")
    wb_sb = wtile("wbh", [128, 2, 2, 128], "(k p) (m j) -> p k m j")
    ac_sb = singles.tile([128, 2, 1], F32R)
    nc.scalar.dma_start(out=ac_sb, in_=t["ach"].rearrange("(k p) o -> p k o", p=128))

    def btile(name):
        sb = singles.tile([128, 2], F32, name=name + "_sb")
        nc.scalar.dma_start(out=sb, in_=t[name].rearrange("(m p) -> p m", p=128))
        return sb

    b1_sb, bv_sb, ba_sb, bbh_sb = btile("b1"), btile("bv"), btile("ba"), btile("bbh")
    acb_sb = singles.tile([1, 1], F32)
    nc.scalar.dma_start(out=acb_sb, in_=t["acb"][None, :])

    s_parts = singles.tile([128, 2, nblocks], F32)
    z_parts = singles.tile([1, nblocks], F32)

    # xt is host-packed as [128, nblocks*8*NB]: partition p holds, per block,
    # 8 contiguous 2KB runs (one per 128-feature chunk) -> 16KB/partition DMA
    # lines at full HBM line rate.  Padded tail columns are never read.
    for b in range(nblocks):
        n0 = b * NB
        nb = min(NB, npc - n0)

        if b == 0:
            x_tile = x_tiles0
        else:
            x_tile = xpool.tile([128, 8, NB], F32R, tag="x")
            nc.sync.dma_start(
                out=x_tile,
                in_=t["xt"][:, b * 8 * NB : (b + 1) * 8 * NB].rearrange("p (c j) -> p c j", j=NB),
            )

        # h^T = relu(W1^T x^T + b1)   (PE f32r hi+lo, DVE bias+relu)
        h_sb = actp.tile([128, 2, nb], F32R, tag="h")
        for m in range(2):
            ph = psum.tile([128, nb], F32, tag="ph")
            nmm = 8 * len(w1_parts)
            i = 0
            for c in range(8):
                for w1p in w1_parts:
                    nc.tensor.matmul(ph, w1p[:, c, m, :], x_tile[:, c, :nb], start=(i == 0), stop=(i == nmm - 1))
                    i += 1
            nc.vector.tensor_scalar(out=h_sb[:, m, :], in0=ph, scalar1=b1_sb[:, m : m + 1],
                                    scalar2=0.0, op0=ALU.add, op1=ALU.max)

        # f^T = Wv^T h^T + bv  (the reference's 'fused' == v)
        f_sb = actp.tile([128, 2, nb], F32R, tag="f")
        for m in range(2):
            pv = psum.tile([128, nb], F32, tag="pv")
            nmm = 2 * len(wv_parts)
            i = 0
            for k in range(2):
                for wvp in wv_parts:
                    nc.tensor.matmul(pv, wvp[:, k, m, :], h_sb[:, k, :], start=(i == 0), stop=(i == nmm - 1))
                    i += 1
            nc.scalar.activation(out=f_sb[:, m, :], in_=pv, func=AF.Identity, bias=bv_sb[:, m : m + 1], scale=1.0)

        # a^T = tanh(Wa^T f^T + ba);  t^T = tanh((Wb^T f^T + bb)/2)
        a_sb = actp.tile([128, 2, nb], F32R, tag="a")
        for m in range(2):
            pg1 = psum.tile([128, nb], F32, tag="pg1")
            for k in range(2):
                nc.tensor.matmul(pg1, wa_sb[:, k, m, :], f_sb[:, k, :], start=(k == 0), stop=(k == 1))
            nc.scalar.activation(out=a_sb[:, m, :], in_=pg1, func=AF.Tanh, bias=ba_sb[:, m : m + 1], scale=1.0)
        bt_sb = actp.tile([128, 2, nb], F32R, tag="bt")
        for m in range(2):
            pg2 = psum.tile([128, nb], F32, tag="pg2")
            for k in range(2):
                nc.tensor.matmul(pg2, wb_sb[:, k, m, :], f_sb[:, k, :], start=(k == 0), stop=(k == 1))
            nc.scalar.activation(out=bt_sb[:, m, :], in_=pg2, func=AF.Tanh, bias=bbh_sb[:, m : m + 1], scale=0.5)

        # g' = a * (1 + t)   (sigmoid trick; the 0.5 lives in ach)
        g_sb = actp.tile([128, 2, nb], F32R, tag="g")
        for m in range(2):
            nc.vector.scalar_tensor_tensor(out=g_sb[:, m, :], in0=bt_sb[:, m, :], scalar=1.0,
                                           in1=a_sb[:, m, :], op0=ALU.add, op1=ALU.mult)

        # A = g' @ (0.5 ac_w)  -> (1, nb);  w = exp(A + ac_b); Z += sum(w)
        pA = psum.tile([1, nb], F32, tag="pg1")
        for k in range(2):
            nc.tensor.matmul(pA, ac_sb[:, k, :], g_sb[:, k, :], start=(k == 0), stop=(k == 1))
        w_sb = actp.tile([1, nb], F32R, tag="w")
        nc.scalar.activation(out=w_sb, in_=pA, func=AF.Exp, bias=acb_sb[0:1, 0:1], scale=1.0,
                             accum_out=z_parts[:, b : b + 1])

        # broadcast w to all partitions (GpSimd), then S[:,m,b] = rowsum(f * w)
        wb_bc = actp.tile([128, nb], F32R, tag="wb")
        nc.gpsimd.partition_broadcast(wb_bc, w_sb)
        for m in range(2):
            wf = actp.tile([128, nb], F32, tag="wf")
            nc.vector.scalar_tensor_tensor(out=wf, in0=f_sb[:, m, :], scalar=0.0, in1=wb_bc,
                                           op0=ALU.add, op1=ALU.mult,
                                           accum_out=s_parts[:, m, b : b + 1])

    nc.sync.dma_start(out=t["s_out"], in_=s_parts)
    nc.sync.dma_start(out=t["z_out"], in_=z_parts)


def build_program(npc: int = NPC, split=SPLIT, enable_asserts: bool = False):
    nblocks = (npc + NB - 1) // NB
    nc = bacc.Bacc("TRN2", target_bir_lowering=False, debug=False, enable_asserts=enable_asserts)

    t = {}
    t["xt"] = nc.dram_tensor("xt", [128, ((npc + NB - 1) // NB) * 8 * NB], F32R, kind="ExternalInput").ap()
    names = [("w1h", [D_IN, D_HID]), ("wvh", [D_HID, D_HID]), ("wah", [D_HID, D_HID]),
             ("wbh", [D_HID, D_HID]), ("ach", [D_HID, 1])]
    if "w1" in split:
        names.append(("w1l", [D_IN, D_HID]))
    if "wv" in split:
        names.append(("wvl", [D_HID, D_HID]))
    for nm, shp in names:
        t[nm] = nc.dram_tensor(nm, shp, F32R, kind="ExternalInput").ap()
    for nm in ("b1", "bv", "ba", "bbh"):
        t[nm] = nc.dram_tensor(nm, [D_HID], F32, kind="ExternalInput").ap()
    t["acb"] = nc.dram_tensor("acb", [1], F32, kind="ExternalInput").ap()
    t["s_out"] = nc.dram_tensor("s_out", [128, 2, nblocks], F32, kind="ExternalOutput").ap()
    t["z_out"] = nc.dram_tensor("z_out", [1, nblocks], F32, kind="ExternalOutput").ap()

    with tile.TileContext(nc) as tc, ExitStack() as ctx:
        _build_tile_kernel(ctx, tc, t, npc, nblocks, split)
    nc.compile()
    return nc


def make_weight_map(inputs, split=SPLIT):
    w1 = np.asarray(inputs["wsi_w"], np.float32)
    wv = np.asarray(inputs["wv_w"], np.float32)
    m = {
        "wah": rne11(inputs["aa_w"]),
        "wbh": rne11(inputs["ab_w"]),
        "ach": rne11(0.5 * np.asarray(inputs["ac_w"], np.float32)),
        "b1": np.asarray(inputs["wsi_b"], np.float32),
        "bv": np.asarray(inputs["wv_b"], np.float32),
        "ba": np.asarray(inputs["aa_b"], np.float32),
        "bbh": 0.5 * np.asarray(inputs["ab_b"], np.float32),
        "acb": np.asarray(inputs["ac_b"], np.float32),
    }
    m["w1h"] = rne11(w1)
    if "w1" in split:
        m["w1l"] = rne11(w1 - m["w1h"])
    m["wvh"] = rne11(wv)
    if "wv" in split:
        m["wvl"] = rne11(wv - m["wvh"])
    return m


def make_in_maps(x_path, weights, npc: int = NPC, n_cores: int = N_CORES):
    x = np.asarray(x_path[0], np.float32)  # (N, 1024)
    nblocks = (npc + NB - 1) // NB
    npad = nblocks * NB
    in_maps = []
    for c in range(n_cores):
        xt = np.zeros((D_IN, npad), np.float32)
        xt[:, :npc] = x[c * npc : (c + 1) * npc].T
        # [ (c8 p128), (b nb) ] -> [ p, (b c8 nb) ]
        packed = np.ascontiguousarray(
            xt.reshape(8, 128, nblocks, NB).transpose(1, 2, 0, 3).reshape(128, nblocks * 8 * NB)
        )
        in_maps.append({"xt": rne11(packed), **weights})
    return in_maps


def finalize(results, c1_w, c1_b, c2_w, c2_b):
    """Host-side reduction of per-core partials + the tiny classifier."""
    S = np.zeros((128, 2), np.float64)
    Z = 0.0
    for r in results:
        S += r["s_out"].sum(axis=-1, dtype=np.float64)
        Z += float(r["z_out"].sum(dtype=np.float64))
    s_vec = S.T.reshape(256)  # feature = m*128 + p
    pooled = (s_vec / Z).astype(np.float32)
    risk = np.maximum(pooled @ np.asarray(c1_w, np.float32) + c1_b, 0.0) @ np.asarray(c2_w, np.float32) + c2_b
    return risk[None, :].astype(np.float32)


_CACHED_NC = None


def kernel(**inputs) -> np.ndarray:
    global _CACHED_NC
    if _CACHED_NC is None:
        _CACHED_NC = build_program()
    nc = _CACHED_NC

    weights = make_weight_map(inputs)
    in_maps = make_in_maps(np.asarray(inputs["x_path"]), weights)
    res = run_bass_kernel_spmd(nc, in_maps, list(range(N_CORES)))
    return finalize(
        res.results,
        np.asarray(inputs["c1_w"], np.float32),
        np.asarray(inputs["c1_b"], np.float32),
        np.asarray(inputs["c2_w"], np.float32),
        np.asarray(inputs["c2_b"], np.float32),
    )



# revision 2
# speedup vs baseline: 1.1112x; 1.1112x over previous
"""Trainium2 Bass kernel for the MCAT gated-attention MIL pooling model.

Math (reference after dead-code elimination + algebraic folding):
  The per-instance cross-attention softmax is over a length-1 axis -> attn_w == 1,
  so fused = v = h @ Wv + bv with h = relu(x_path @ W1 + b1).  The x_cell / wq /
  wk branch is dead.

  Key folding: f( = v) is LINEAR in h, so
    - gating:  f @ Wa = h @ (Wv Wa) + (bv Wa)   -> composed weights on the host
    - pooling: sum_n w_n f_n = (sum_n w_n h_n) @ Wv + bv * sum_n w_n
  The device therefore never materializes f at all:
      h   = relu(x @ W1 + b1)                  (N, 256)
      a   = tanh(h @ Wa' + ba')                Wa' = Wv Wa,      ba' = bv Wa + ba
      t   = tanh(h @ Wb' + bb')                Wb' = 0.5 Wv Wb,  bb' = 0.5 (bv Wb + bb)
      A   = (a * (1 + t)) @ (0.5 ac)           (sigmoid(y) = 0.5 (1 + tanh(y/2)))
      w   = exp(A)          (the ac_b bias cancels in S/Z and is dropped)
      S  += w_n * h_n ;  Z += w_n              per-core partial sums
  Host: pooled = (S/Z) @ Wv + bv ; risk = relu(pooled @ c1 + b) @ c2 + b2  (fp64).

Precision: rel-err budget is 2e-2; measured host study gives 2.3e-3 with x/W1/h
and the gating weights in fp8(e4m3, power-of-2 scaled) and everything else bf16.
fp8 enables DoubleRow matmuls (2 contraction rows per PE cell) for the dominant
x@W1 (8 MMs/block instead of 16) and the gating projections (2 each instead of 4).
Scales are powers of two folded into ACT/DVE epilogues (exact).

Sharding: rows split across 8 cores (6250 each); cores return per-block partial
sums S (128,2,NB) and Z (1,NB); host reduces in fp64 + tiny classifier.
"""

import sys
from contextlib import ExitStack

import numpy as np

try:
    import concourse  # noqa: F401
except ImportError:  # pragma: no cover - fresh grading env
    sys.path.insert(0, "/opt/trn_rl_repo")

import ml_dtypes

import concourse.bass as bass
import concourse.tile as tile
from concourse import bacc, mybir
from concourse.bass_utils import run_bass_kernel_spmd

N_CORES = 8
N = 50000
NPC = N // N_CORES  # 6250 rows per core
D_IN = 1024
D_HID = 256
NB = 512  # instances per block (one PSUM bank of fp32)
USE_DR = True  # DoubleRow fp8 matmuls (2 contraction rows/cell)

F32 = mybir.dt.float32
BF16 = mybir.dt.bfloat16
FP8 = mybir.dt.float8e4
AF = mybir.ActivationFunctionType
ALU = mybir.AluOpType
DR = mybir.MatmulPerfMode.DoubleRow

NP_FP8 = ml_dtypes.float8_e4m3
NP_BF16 = ml_dtypes.bfloat16

# power-of-2 quantization scales (folded back out in on-chip epilogues)
S_X = 16.0
S_W1 = 1024.0
S_H = 32.0
S_WAB = 4096.0
SC_H = S_H / (S_X * S_W1)  # psum -> h units
SC_AT = 1.0 / (S_H * S_WAB)  # gating psum -> pre-activation units


def _build_tile_kernel(ctx: ExitStack, tc: tile.TileContext, t, npc, nblocks, zero_bias):
    nc = tc.nc

    singles = ctx.enter_context(tc.tile_pool(name="singles", bufs=1))
    xpool = ctx.enter_context(tc.tile_pool(name="xp", bufs=5))
    actp = ctx.enter_context(tc.tile_pool(name="actp", bufs=3))
    psum = ctx.enter_context(tc.tile_pool(name="psum", bufs=2, space=bass.MemorySpace.PSUM))

    # Block-0 x DMA first in program order: it is on the PE's critical path
    # (weights ride a separate HWDGE ring and overlap it).
    x_tiles0 = xpool.tile([128, 4, 2, NB], FP8, tag="x")
    nc.sync.dma_start(
        out=x_tiles0,
        in_=t["xt"][:, 0 : 8 * NB].rearrange("p (c i j) -> p c i j", i=2, j=NB),
    )

    # ---- persistent weights / biases in SBUF --------------------------------
    w1_sb = singles.tile([128, 4, 2, 2, 128], FP8, name="w1_sb")
    nc.scalar.dma_start(out=w1_sb, in_=t["w1q"].rearrange("p (c i m j) -> p c i m j", i=2, m=2, j=128))
    wa_sb = singles.tile([128, 2, 2, 128], FP8, name="wa_sb")
    nc.scalar.dma_start(out=wa_sb, in_=t["waq"].rearrange("p (i m j) -> p i m j", m=2, j=128))
    wb_sb = singles.tile([128, 2, 2, 128], FP8, name="wb_sb")
    nc.scalar.dma_start(out=wb_sb, in_=t["wbq"].rearrange("p (i m j) -> p i m j", m=2, j=128))
    ac_sb = singles.tile([128, 2, 1], BF16, name="ac_sb")
    nc.scalar.dma_start(out=ac_sb, in_=t["ach"].rearrange("p (k o) -> p k o", o=1))

    if not zero_bias:
        b1_sb = singles.tile([128, 2], F32, name="b1_sb")
        nc.scalar.dma_start(out=b1_sb, in_=t["b1s"].rearrange("(m p) -> p m", p=128))
        ba_sb = singles.tile([128, 2], F32, name="ba_sb")
        nc.scalar.dma_start(out=ba_sb, in_=t["bas"].rearrange("(m p) -> p m", p=128))
        bb_sb = singles.tile([128, 2], F32, name="bb_sb")
        nc.scalar.dma_start(out=bb_sb, in_=t["bbs"].rearrange("(m p) -> p m", p=128))

    s_parts = singles.tile([128, 2, nblocks], F32)
    z_parts = singles.tile([1, nblocks], F32)

    for b in range(nblocks):
        n0 = b * NB
        nb = min(NB, npc - n0)

        if b == 0:
            x_tile = x_tiles0
        else:
            x_tile = xpool.tile([128, 4, 2, NB], FP8, tag="x")
            nc.sync.dma_start(
                out=x_tile,
                in_=t["xt"][:, b * 8 * NB : (b + 1) * 8 * NB].rearrange("p (c i j) -> p c i j", i=2, j=NB),
            )

        # h^T = relu(W1^T x^T + b1), stored as fp8 (scaled by S_H)
        ph = psum.tile([128, 2, NB], F32, tag="ph")
        for m in range(2):
            if USE_DR:
                for c in range(4):
                    nc.tensor.matmul(ph[:, m, :nb], w1_sb[:, c, :, m, :], x_tile[:, c, :, :nb],
                                     perf_mode=DR, start=(c == 0), stop=(c == 3))
            else:
                k = 0
                for c in range(4):
                    for i in range(2):
                        nc.tensor.matmul(ph[:, m, :nb], w1_sb[:, c, i, m, :], x_tile[:, c, i, :nb],
                                         start=(k == 0), stop=(k == 7))
                        k += 1
        h_sb = actp.tile([128, 2, NB], FP8, tag="h")
        if zero_bias:
            nc.vector.tensor_scalar(out=h_sb[:, :, :nb], in0=ph[:, :, :nb], scalar1=SC_H,
                                    scalar2=0.0, op0=ALU.mult, op1=ALU.max)
        else:
            for m in range(2):
                nc.scalar.activation(out=h_sb[:, m, :nb], in_=ph[:, m, :nb], func=AF.Relu,
                                     bias=b1_sb[:, m : m + 1], scale=SC_H)

        # a = tanh(h Wa' + ba');  t = tanh(h Wb' + bb')  (0.5s folded host-side)
        pa = psum.tile([128, 2, NB], F32, tag="pg")
        pt = psum.tile([128, 2, NB], F32, tag="pg")
        for m in range(2):
            if USE_DR:
                nc.tensor.matmul(pa[:, m, :nb], wa_sb[:, :, m, :], h_sb[:, :, :nb], perf_mode=DR)
                nc.tensor.matmul(pt[:, m, :nb], wb_sb[:, :, m, :], h_sb[:, :, :nb], perf_mode=DR)
            else:
                for i in range(2):
                    nc.tensor.matmul(pa[:, m, :nb], wa_sb[:, i, m, :], h_sb[:, i, :nb],
                                     start=(i == 0), stop=(i == 1))
                for i in range(2):
                    nc.tensor.matmul(pt[:, m, :nb], wb_sb[:, i, m, :], h_sb[:, i, :nb],
                                     start=(i == 0), stop=(i == 1))
        a_sb = actp.tile([128, 2, NB], BF16, tag="a")
        t_sb = actp.tile([128, 2, NB], BF16, tag="t")
        if zero_bias:
            nc.scalar.activation(out=a_sb[:, :, :nb], in_=pa[:, :, :nb], func=AF.Tanh, scale=SC_AT)
            nc.scalar.activation(out=t_sb[:, :, :nb], in_=pt[:, :, :nb], func=AF.Tanh, scale=SC_AT)
        else:
            for m in range(2):
                nc.scalar.activation(out=a_sb[:, m, :nb], in_=pa[:, m, :nb], func=AF.Tanh,
                                     bias=ba_sb[:, m : m + 1], scale=SC_AT)
                nc.scalar.activation(out=t_sb[:, m, :nb], in_=pt[:, m, :nb], func=AF.Tanh,
                                     bias=bb_sb[:, m : m + 1], scale=SC_AT)

        # g = a * (1 + t)
        g_sb = actp.tile([128, 2, NB], BF16, tag="g")
        nc.vector.scalar_tensor_tensor(out=g_sb[:, :, :nb], in0=t_sb[:, :, :nb], scalar=1.0,
                                       in1=a_sb[:, :, :nb], op0=ALU.add, op1=ALU.mult)

        # A = g @ (0.5 ac) -> (1, nb);  w = exp(A); Z += sum(w)
        pA = psum.tile([1, NB], F32, tag="pg")
        for k in range(2):
            nc.tensor.matmul(pA[:, :nb], ac_sb[:, k, :], g_sb[:, k, :nb], start=(k == 0), stop=(k == 1))
        w_sb = actp.tile([1, NB], BF16, tag="w")
        nc.scalar.activation(out=w_sb[:, :nb], in_=pA[:, :nb], func=AF.Exp, scale=1.0,
                             accum_out=z_parts[:, b : b + 1])

        # broadcast w to all partitions (GpSimd), then S[:,m,b] += rowsum(h/S_H * w)
        wb_bc = actp.tile([128, NB], BF16, tag="wb")
        nc.gpsimd.partition_broadcast(wb_bc[:, :nb], w_sb[:, :nb])
        for m in range(2):
            wf = actp.tile([128, NB], BF16, tag="wf")
            nc.vector.scalar_tensor_tensor(out=wf[:, :nb], in0=h_sb[:, m, :nb], scalar=1.0 / S_H,
                                           in1=wb_bc[:, :nb], op0=ALU.mult, op1=ALU.mult,
                                           accum_out=s_parts[:, m, b : b + 1])

    nc.sync.dma_start(out=t["s_out"], in_=s_parts)
    nc.sync.dma_start(out=t["z_out"], in_=z_parts)


def build_program(npc: int = NPC, zero_bias: bool = True, enable_asserts: bool = False):
    nblocks = (npc + NB - 1) // NB
    nc = bacc.Bacc("TRN2", target_bir_lowering=False, debug=False, enable_asserts=enable_asserts)

    t = {}
    t["xt"] = nc.dram_tensor("xt", [128, nblocks * 8 * NB], FP8, kind="ExternalInput").ap()
    t["w1q"] = nc.dram_tensor("w1q", [128, 2048], FP8, kind="ExternalInput").ap()
    t["waq"] = nc.dram_tensor("waq", [128, 512], FP8, kind="ExternalInput").ap()
    t["wbq"] = nc.dram_tensor("wbq", [128, 512], FP8, kind="ExternalInput").ap()
    t["ach"] = nc.dram_tensor("ach", [128, 2], BF16, kind="ExternalInput").ap()
    if not zero_bias:
        for nm in ("b1s", "bas", "bbs"):
            t[nm] = nc.dram_tensor(nm, [D_HID], F32, kind="ExternalInput").ap()
    t["s_out"] = nc.dram_tensor("s_out", [128, 2, nblocks], F32, kind="ExternalOutput").ap()
    t["z_out"] = nc.dram_tensor("z_out", [1, nblocks], F32, kind="ExternalOutput").ap()

    with tile.TileContext(nc) as tc, ExitStack() as ctx:
        _build_tile_kernel(ctx, tc, t, npc, nblocks, zero_bias)
    nc.compile()
    return nc


def _q8(a: np.ndarray, scale: float) -> np.ndarray:
    return np.ascontiguousarray((np.asarray(a, np.float32) * scale).astype(NP_FP8))


def make_weight_map(inputs, zero_bias=None):
    W1 = np.asarray(inputs["wsi_w"], np.float64)
    Wv = np.asarray(inputs["wv_w"], np.float64)
    Wa = np.asarray(inputs["aa_w"], np.float64)
    Wb = np.asarray(inputs["ab_w"], np.float64)
    ac = np.asarray(inputs["ac_w"], np.float64)
    bv = np.asarray(inputs["wv_b"], np.float64)
    b1 = np.asarray(inputs["wsi_b"], np.float64)
    ba = np.asarray(bv @ Wa + np.asarray(inputs["aa_b"], np.float64))
    bb = np.asarray(0.5 * (bv @ Wb + np.asarray(inputs["ab_b"], np.float64)))

    # composed gating weights (f folded away); 0.5 of the tanh-sigmoid in Wb'
    Wa_c = Wv @ Wa
    Wb_c = 0.5 * (Wv @ Wb)

    # device layouts
    w1q = _q8(W1, S_W1).reshape(4, 2, 128, 2, 128).transpose(2, 0, 1, 3, 4).reshape(128, 2048)
    waq = _q8(Wa_c, S_WAB).reshape(2, 128, 2, 128).transpose(1, 0, 2, 3).reshape(128, 512)
    wbq = _q8(Wb_c, S_WAB).reshape(2, 128, 2, 128).transpose(1, 0, 2, 3).reshape(128, 512)
    ach = np.ascontiguousarray(
        (0.5 * ac).astype(NP_BF16).reshape(2, 128, 1).transpose(1, 0, 2).reshape(128, 2)
    )
    m = {"w1q": np.ascontiguousarray(w1q), "waq": np.ascontiguousarray(waq),
         "wbq": np.ascontiguousarray(wbq), "ach": ach}
    zb = not (np.any(b1) or np.any(ba) or np.any(bb))
    if not zb:
        m["b1s"] = (np.asarray(b1, np.float32) * S_H).astype(np.float32)
        m["bas"] = np.asarray(ba, np.float32)
        m["bbs"] = np.asarray(bb, np.float32)
    m["_zero_bias"] = zb
    return m


def make_in_maps(x_path, weights, npc: int = NPC, n_cores: int = N_CORES):
    x = np.asarray(x_path[0], np.float32)  # (N, 1024)
    nblocks = (npc + NB - 1) // NB
    npad = nblocks * NB
    w = {k: v for k, v in weights.items() if not k.startswith("_")}
    in_maps = []
    for c in range(n_cores):
        xc = np.zeros((npad, D_IN), np.float32)
        xc[:npc] = x[c * npc : (c + 1) * npc]
        xq = (xc * S_X).astype(NP_FP8)
        # [inst, feat] -> [p, (b c i j)] with feat = c*256 + i*128 + p
        packed = np.ascontiguousarray(
            xq.reshape(nblocks, NB, 4, 2, 128).transpose(4, 0, 2, 3, 1).reshape(128, nblocks * 8 * NB)
        )
        in_maps.append({"xt": packed, **w})
    return in_maps


def finalize(results, c1_w, c1_b, c2_w, c2_b, wv_w, wv_b):
    """Host-side reduction of per-core partials, Wv application + classifier."""
    S = np.zeros((128, 2), np.float64)
    Z = 0.0
    for r in results:
        S += np.asarray(r["s_out"], np.float64).sum(axis=-1)
        Z += float(np.asarray(r["z_out"], np.float64).sum())
    s_vec = S.T.reshape(256)  # feature = m*128 + p
    pooled = (s_vec / Z) @ np.asarray(wv_w, np.float64) + np.asarray(wv_b, np.float64)
    risk = (
        np.maximum(pooled @ np.asarray(c1_w, np.float64) + np.asarray(c1_b, np.float64), 0.0)
        @ np.asarray(c2_w, np.float64)
        + np.asarray(c2_b, np.float64)
    )
    return risk[None, :].astype(np.float32)


_CACHED = {}


def kernel(**inputs) -> np.ndarray:
    weights = make_weight_map(inputs)
    zb = weights["_zero_bias"]
    if zb not in _CACHED:
        _CACHED[zb] = build_program(zero_bias=zb)
    nc = _CACHED[zb]

    in_maps = make_in_maps(np.asarray(inputs["x_path"]), weights)
    res = run_bass_kernel_spmd(nc, in_maps, list(range(N_CORES)))
    return finalize(
        res.results,
        inputs["c1_w"], inputs["c1_b"], inputs["c2_w"], inputs["c2_b"],
        inputs["wv_w"], inputs["wv_b"],
    )


# revision 3
# speedup vs baseline: 1.1156x; 1.0040x over previous
"""Trainium2 Bass kernel for the MCAT gated-attention MIL pooling model.

Math (reference after dead-code elimination + algebraic folding):
  The per-instance cross-attention softmax is over a length-1 axis -> attn_w == 1,
  so fused = v = h @ Wv + bv with h = relu(x_path @ W1 + b1).  The x_cell / wq /
  wk branch is dead.

  Key folding: f( = v) is LINEAR in h, so
    - gating:  f @ Wa = h @ (Wv Wa) + (bv Wa)   -> composed weights on the host
    - pooling: sum_n w_n f_n = (sum_n w_n h_n) @ Wv + bv * sum_n w_n
  The device therefore never materializes f at all:
      h   = relu(x @ W1 + b1)                  (N, 256)
      a   = tanh(h @ Wa' + ba')                Wa' = Wv Wa,      ba' = bv Wa + ba
      t   = tanh(h @ Wb' + bb')                Wb' = 0.5 Wv Wb,  bb' = 0.5 (bv Wb + bb)
      A   = (a * (1 + t)) @ (0.5 ac)           (sigmoid(y) = 0.5 (1 + tanh(y/2)))
      w   = exp(A)          (the ac_b bias cancels in S/Z and is dropped)
      S  += w_n * h_n ;  Z += w_n              per-core partial sums
  Host: pooled = (S/Z) @ Wv + bv ; risk = relu(pooled @ c1 + b) @ c2 + b2  (fp64).

Precision: rel-err budget is 2e-2; measured host study gives 2.3e-3 with x/W1/h
and the gating weights in fp8(e4m3, power-of-2 scaled) and everything else bf16.
fp8 enables DoubleRow matmuls (2 contraction rows per PE cell) for the dominant
x@W1 (8 MMs/block instead of 16) and the gating projections (2 each instead of 4).
Scales are powers of two folded into ACT/DVE epilogues (exact).

Sharding: rows split across 8 cores (6250 each); cores return per-block partial
sums S (128,2,NB) and Z (1,NB); host reduces in fp64 + tiny classifier.
"""

import sys
from contextlib import ExitStack

import numpy as np

try:
    import concourse  # noqa: F401
except ImportError:  # pragma: no cover - fresh grading env
    sys.path.insert(0, "/opt/trn_rl_repo")

import ml_dtypes

import concourse.bass as bass
import concourse.tile as tile
from concourse import bacc, mybir
from concourse.bass_utils import run_bass_kernel_spmd

N_CORES = 8
N = 50000
NPC = N // N_CORES  # 6250 rows per core
D_IN = 1024
D_HID = 256
NB = 512  # instances per block (one PSUM bank of fp32)
USE_DR = True  # DoubleRow fp8 matmuls (2 contraction rows/cell)

F32 = mybir.dt.float32
BF16 = mybir.dt.bfloat16
FP8 = mybir.dt.float8e4
AF = mybir.ActivationFunctionType
ALU = mybir.AluOpType
DR = mybir.MatmulPerfMode.DoubleRow

NP_FP8 = ml_dtypes.float8_e4m3
NP_BF16 = ml_dtypes.bfloat16

# power-of-2 quantization scales (folded back out in on-chip epilogues)
S_X = 16.0
S_W1 = 1024.0
S_H = 32.0
S_WAB = 4096.0
SC_H = S_H / (S_X * S_W1)  # psum -> h units
SC_AT = 1.0 / (S_H * S_WAB)  # gating psum -> pre-activation units


def _build_tile_kernel(ctx: ExitStack, tc: tile.TileContext, t, npc, nblocks, zero_bias):
    nc = tc.nc

    singles = ctx.enter_context(tc.tile_pool(name="singles", bufs=1))
    xpool = ctx.enter_context(tc.tile_pool(name="xp", bufs=5))
    actp = ctx.enter_context(tc.tile_pool(name="actp", bufs=3))
    psum = ctx.enter_context(tc.tile_pool(name="psum", bufs=2, space=bass.MemorySpace.PSUM))

    # Block-0 x DMA first in program order: it is on the PE's critical path
    # (weights ride a separate HWDGE ring and overlap it).
    x_tiles0 = xpool.tile([128, 4, 2, NB], FP8, tag="x")
    nc.sync.dma_start(
        out=x_tiles0,
        in_=t["xt"][:, 0 : 8 * NB].rearrange("p (c i j) -> p c i j", i=2, j=NB),
    )

    # ---- persistent weights / biases in SBUF --------------------------------
    w1_sb = singles.tile([128, 4, 2, 2, 128], FP8, name="w1_sb")
    nc.scalar.dma_start(out=w1_sb, in_=t["w1q"].rearrange("p (c i m j) -> p c i m j", i=2, m=2, j=128))
    wa_sb = singles.tile([128, 2, 2, 128], FP8, name="wa_sb")
    nc.scalar.dma_start(out=wa_sb, in_=t["waq"].rearrange("p (i m j) -> p i m j", m=2, j=128))
    wb_sb = singles.tile([128, 2, 2, 128], FP8, name="wb_sb")
    nc.scalar.dma_start(out=wb_sb, in_=t["wbq"].rearrange("p (i m j) -> p i m j", m=2, j=128))
    ac_sb = singles.tile([128, 2, 1], BF16, name="ac_sb")
    nc.scalar.dma_start(out=ac_sb, in_=t["ach"].rearrange("p (k o) -> p k o", o=1))

    if not zero_bias:
        b1_sb = singles.tile([128, 2], F32, name="b1_sb")
        nc.scalar.dma_start(out=b1_sb, in_=t["b1s"].rearrange("(m p) -> p m", p=128))
        ba_sb = singles.tile([128, 2], F32, name="ba_sb")
        nc.scalar.dma_start(out=ba_sb, in_=t["bas"].rearrange("(m p) -> p m", p=128))
        bb_sb = singles.tile([128, 2], F32, name="bb_sb")
        nc.scalar.dma_start(out=bb_sb, in_=t["bbs"].rearrange("(m p) -> p m", p=128))

    s_parts = singles.tile([128, 2, nblocks], F32)
    z_parts = singles.tile([1, nblocks], F32)

    h_tiles = {}

    def h_phase(b):
        nb = min(NB, npc - b * NB)
        if b == 0:
            x_tile = x_tiles0
        else:
            x_tile = xpool.tile([128, 4, 2, NB], FP8, tag="x")
            nc.sync.dma_start(
                out=x_tile,
                in_=t["xt"][:, b * 8 * NB : (b + 1) * 8 * NB].rearrange("p (c i j) -> p c i j", i=2, j=NB),
            )

        # h^T = relu(W1^T x^T + b1), stored as fp8 (scaled by S_H)
        ph = psum.tile([128, 2, NB], F32, tag="ph")
        for m in range(2):
            if USE_DR:
                for c in range(4):
                    nc.tensor.matmul(ph[:, m, :nb], w1_sb[:, c, :, m, :], x_tile[:, c, :, :nb],
                                     perf_mode=DR, start=(c == 0), stop=(c == 3))
            else:
                k = 0
                for c in range(4):
                    for i in range(2):
                        nc.tensor.matmul(ph[:, m, :nb], w1_sb[:, c, i, m, :], x_tile[:, c, i, :nb],
                                         start=(k == 0), stop=(k == 7))
                        k += 1
        h_sb = actp.tile([128, 2, NB], FP8, tag="h", bufs=4)
        if zero_bias:
            nc.vector.tensor_scalar(out=h_sb[:, :, :nb], in0=ph[:, :, :nb], scalar1=SC_H,
                                    scalar2=0.0, op0=ALU.mult, op1=ALU.max)
        else:
            for m in range(2):
                nc.scalar.activation(out=h_sb[:, m, :nb], in_=ph[:, m, :nb], func=AF.Relu,
                                     bias=b1_sb[:, m : m + 1], scale=SC_H)
        h_tiles[b] = h_sb

    def gate_phase(b):
        nb = min(NB, npc - b * NB)
        h_sb = h_tiles.pop(b)

        # a = tanh(h Wa' + ba');  t = tanh(h Wb' + bb')  (0.5s folded host-side)
        pa = psum.tile([128, 2, NB], F32, tag="pg")
        pt = psum.tile([128, 2, NB], F32, tag="pg")
        for m in range(2):
            if USE_DR:
                nc.tensor.matmul(pa[:, m, :nb], wa_sb[:, :, m, :], h_sb[:, :, :nb], perf_mode=DR)
                nc.tensor.matmul(pt[:, m, :nb], wb_sb[:, :, m, :], h_sb[:, :, :nb], perf_mode=DR)
            else:
                for i in range(2):
                    nc.tensor.matmul(pa[:, m, :nb], wa_sb[:, i, m, :], h_sb[:, i, :nb],
                                     start=(i == 0), stop=(i == 1))
                for i in range(2):
                    nc.tensor.matmul(pt[:, m, :nb], wb_sb[:, i, m, :], h_sb[:, i, :nb],
                                     start=(i == 0), stop=(i == 1))
        a_sb = actp.tile([128, 2, NB], BF16, tag="a")
        t_sb = actp.tile([128, 2, NB], BF16, tag="t")
        if zero_bias:
            nc.scalar.activation(out=a_sb[:, :, :nb], in_=pa[:, :, :nb], func=AF.Tanh, scale=SC_AT)
            nc.scalar.activation(out=t_sb[:, :, :nb], in_=pt[:, :, :nb], func=AF.Tanh, scale=SC_AT)
        else:
            for m in range(2):
                nc.scalar.activation(out=a_sb[:, m, :nb], in_=pa[:, m, :nb], func=AF.Tanh,
                                     bias=ba_sb[:, m : m + 1], scale=SC_AT)
                nc.scalar.activation(out=t_sb[:, m, :nb], in_=pt[:, m, :nb], func=AF.Tanh,
                                     bias=bb_sb[:, m : m + 1], scale=SC_AT)

        # g = a * (1 + t)
        g_sb = actp.tile([128, 2, NB], BF16, tag="g")
        nc.vector.scalar_tensor_tensor(out=g_sb[:, :, :nb], in0=t_sb[:, :, :nb], scalar=1.0,
                                       in1=a_sb[:, :, :nb], op0=ALU.add, op1=ALU.mult)

        # A = g @ (0.5 ac) -> (1, nb);  w = exp(A); Z += sum(w)
        pA = psum.tile([1, NB], F32, tag="pg")
        for k in range(2):
            nc.tensor.matmul(pA[:, :nb], ac_sb[:, k, :], g_sb[:, k, :nb], start=(k == 0), stop=(k == 1))
        w_sb = actp.tile([1, NB], BF16, tag="w")
        nc.scalar.activation(out=w_sb[:, :nb], in_=pA[:, :nb], func=AF.Exp, scale=1.0,
                             accum_out=z_parts[:, b : b + 1])

        # broadcast w to all partitions (GpSimd), then S[:,m,b] += rowsum(h/S_H * w)
        wb_bc = actp.tile([128, NB], BF16, tag="wb")
        nc.gpsimd.partition_broadcast(wb_bc[:, :nb], w_sb[:, :nb])
        for m in range(2):
            wf = actp.tile([128, NB], BF16, tag="wf")
            nc.vector.scalar_tensor_tensor(out=wf[:, :nb], in0=h_sb[:, m, :nb], scalar=1.0 / S_H,
                                           in1=wb_bc[:, :nb], op0=ALU.mult, op1=ALU.mult,
                                           accum_out=s_parts[:, m, b : b + 1])

    # Software pipeline: the gating phase of block b is emitted two blocks
    # late so the per-engine FIFOs never stall on the cross-engine chain
    # (relu -> gate MMs -> tanh -> g -> A -> exp -> bcast -> weighted sum)
    # and the PE stays continuously busy (HAM stays warm).
    LAG = 2
    for b in range(nblocks):
        h_phase(b)
        if b >= LAG:
            gate_phase(b - LAG)
    for b in range(max(0, nblocks - LAG), nblocks):
        gate_phase(b)

    nc.sync.dma_start(out=t["s_out"], in_=s_parts)
    nc.sync.dma_start(out=t["z_out"], in_=z_parts)


def build_program(npc: int = NPC, zero_bias: bool = True, enable_asserts: bool = False):
    nblocks = (npc + NB - 1) // NB
    nc = bacc.Bacc("TRN2", target_bir_lowering=False, debug=False, enable_asserts=enable_asserts)

    t = {}
    t["xt"] = nc.dram_tensor("xt", [128, nblocks * 8 * NB], FP8, kind="ExternalInput").ap()
    t["w1q"] = nc.dram_tensor("w1q", [128, 2048], FP8, kind="ExternalInput").ap()
    t["waq"] = nc.dram_tensor("waq", [128, 512], FP8, kind="ExternalInput").ap()
    t["wbq"] = nc.dram_tensor("wbq", [128, 512], FP8, kind="ExternalInput").ap()
    t["ach"] = nc.dram_tensor("ach", [128, 2], BF16, kind="ExternalInput").ap()
    if not zero_bias:
        for nm in ("b1s", "bas", "bbs"):
            t[nm] = nc.dram_tensor(nm, [D_HID], F32, kind="ExternalInput").ap()
    t["s_out"] = nc.dram_tensor("s_out", [128, 2, nblocks], F32, kind="ExternalOutput").ap()
    t["z_out"] = nc.dram_tensor("z_out", [1, nblocks], F32, kind="ExternalOutput").ap()

    with tile.TileContext(nc) as tc, ExitStack() as ctx:
        _build_tile_kernel(ctx, tc, t, npc, nblocks, zero_bias)
    nc.compile()
    return nc


def _q8(a: np.ndarray, scale: float) -> np.ndarray:
    return np.ascontiguousarray((np.asarray(a, np.float32) * scale).astype(NP_FP8))


def make_weight_map(inputs, zero_bias=None):
    W1 = np.asarray(inputs["wsi_w"], np.float64)
    Wv = np.asarray(inputs["wv_w"], np.float64)
    Wa = np.asarray(inputs["aa_w"], np.float64)
    Wb = np.asarray(inputs["ab_w"], np.float64)
    ac = np.asarray(inputs["ac_w"], np.float64)
    bv = np.asarray(inputs["wv_b"], np.float64)
    b1 = np.asarray(inputs["wsi_b"], np.float64)
    ba = np.asarray(bv @ Wa + np.asarray(inputs["aa_b"], np.float64))
    bb = np.asarray(0.5 * (bv @ Wb + np.asarray(inputs["ab_b"], np.float64)))

    # composed gating weights (f folded away); 0.5 of the tanh-sigmoid in Wb'
    Wa_c = Wv @ Wa
    Wb_c = 0.5 * (Wv @ Wb)

    # device layouts
    w1q = _q8(W1, S_W1).reshape(4, 2, 128, 2, 128).transpose(2, 0, 1, 3, 4).reshape(128, 2048)
    waq = _q8(Wa_c, S_WAB).reshape(2, 128, 2, 128).transpose(1, 0, 2, 3).reshape(128, 512)
    wbq = _q8(Wb_c, S_WAB).reshape(2, 128, 2, 128).transpose(1, 0, 2, 3).reshape(128, 512)
    ach = np.ascontiguousarray(
        (0.5 * ac).astype(NP_BF16).reshape(2, 128, 1).transpose(1, 0, 2).reshape(128, 2)
    )
    m = {"w1q": np.ascontiguousarray(w1q), "waq": np.ascontiguousarray(waq),
         "wbq": np.ascontiguousarray(wbq), "ach": ach}
    zb = not (np.any(b1) or np.any(ba) or np.any(bb))
    if not zb:
        m["b1s"] = (np.asarray(b1, np.float32) * S_H).astype(np.float32)
        m["bas"] = np.asarray(ba, np.float32)
        m["bbs"] = np.asarray(bb, np.float32)
    m["_zero_bias"] = zb
    return m


def make_in_maps(x_path, weights, npc: int = NPC, n_cores: int = N_CORES):
    x = np.asarray(x_path[0], np.float32)  # (N, 1024)
    nblocks = (npc + NB - 1) // NB
    npad = nblocks * NB
    w = {k: v for k, v in weights.items() if not k.startswith("_")}
    in_maps = []
    for c in range(n_cores):
        xc = np.zeros((npad, D_IN), np.float32)
        xc[:npc] = x[c * npc : (c + 1) * npc]
        xq = (xc * S_X).astype(NP_FP8)
        # [inst, feat] -> [p, (b c i j)] with feat = c*256 + i*128 + p
        packed = np.ascontiguousarray(
            xq.reshape(nblocks, NB, 4, 2, 128).transpose(4, 0, 2, 3, 1).reshape(128, nblocks * 8 * NB)
        )
        in_maps.append({"xt": packed, **w})
    return in_maps


def finalize(results, c1_w, c1_b, c2_w, c2_b, wv_w, wv_b):
    """Host-side reduction of per-core partials, Wv application + classifier."""
    S = np.zeros((128, 2), np.float64)
    Z = 0.0
    for r in results:
        S += np.asarray(r["s_out"], np.float64).sum(axis=-1)
        Z += float(np.asarray(r["z_out"], np.float64).sum())
    s_vec = S.T.reshape(256)  # feature = m*128 + p
    pooled = (s_vec / Z) @ np.asarray(wv_w, np.float64) + np.asarray(wv_b, np.float64)
    risk = (
        np.maximum(pooled @ np.asarray(c1_w, np.float64) + np.asarray(c1_b, np.float64), 0.0)
        @ np.asarray(c2_w, np.float64)
        + np.asarray(c2_b, np.float64)
    )
    return risk[None, :].astype(np.float32)


_CACHED = {}


def kernel(**inputs) -> np.ndarray:
    weights = make_weight_map(inputs)
    zb = weights["_zero_bias"]
    if zb not in _CACHED:
        _CACHED[zb] = build_program(zero_bias=zb)
    nc = _CACHED[zb]

    in_maps = make_in_maps(np.asarray(inputs["x_path"]), weights)
    res = run_bass_kernel_spmd(nc, in_maps, list(range(N_CORES)))
    return finalize(
        res.results,
        inputs["c1_w"], inputs["c1_b"], inputs["c2_w"], inputs["c2_b"],
        inputs["wv_w"], inputs["wv_b"],
    )


# revision 5
# speedup vs baseline: 1.7175x; 1.5395x over previous
"""Trainium2 Bass kernel for the MCAT gated-attention MIL pooling model.

Math (reference after dead-code elimination + algebraic folding):
  The per-instance cross-attention softmax is over a length-1 axis -> attn_w == 1,
  so fused = v = h @ Wv + bv with h = relu(x_path @ W1 + b1).  The x_cell / wq /
  wk branch is dead.

  Key folding: f( = v) is LINEAR in h, so
    - gating:  f @ Wa = h @ (Wv Wa) + (bv Wa)   -> composed weights on the host
    - pooling: sum_n w_n f_n = (sum_n w_n h_n) @ Wv + bv * sum_n w_n
  The device therefore never materializes f at all:
      h   = relu(x @ W1 + b1)                  (N, 256)
      a   = tanh(h @ Wa' + ba')                Wa' = Wv Wa,      ba' = bv Wa + ba
      t   = tanh(h @ Wb' + bb')                Wb' = 0.5 Wv Wb,  bb' = 0.5 (bv Wb + bb)
      A   = (a * (1 + t)) @ (0.5 ac)           (sigmoid(y) = 0.5 (1 + tanh(y/2)))
      w   = exp(A)          (the ac_b bias cancels in S/Z and is dropped)
      S  += w_n * h_n ;  Z += w_n              per-core partial sums
  Host: pooled = (S/Z) @ Wv + bv ; risk = relu(pooled @ c1 + b) @ c2 + b2  (fp64).

Precision: rel-err budget is 2e-2; measured host study gives 2.3e-3 with x/W1/h
and the gating weights in fp8(e4m3, power-of-2 scaled) and everything else bf16.
fp8 enables DoubleRow matmuls (2 contraction rows per PE cell) for the dominant
x@W1 (8 MMs/block instead of 16) and the gating projections (2 each instead of 4).
Scales are powers of two folded into ACT/DVE epilogues (exact).

Sharding: rows split across 8 cores (6250 each); cores return per-block partial
sums S (128,2,NB) and Z (1,NB); host reduces in fp64 + tiny classifier.
"""

import sys
from contextlib import ExitStack

import numpy as np

try:
    import concourse  # noqa: F401
except ImportError:  # pragma: no cover - fresh grading env
    sys.path.insert(0, "/opt/trn_rl_repo")

import ml_dtypes

import concourse.bass as bass
import concourse.tile as tile
from concourse import bacc, mybir
from concourse.bass_utils import run_bass_kernel_spmd

N_CORES = 8
N = 50000
NPC = N // N_CORES  # 6250 rows per core
D_IN = 1024
D_HID = 256
NB = 512  # instances per block (one PSUM bank of fp32)
USE_DR = True  # DoubleRow fp8 matmuls (2 contraction rows/cell)

F32 = mybir.dt.float32
BF16 = mybir.dt.bfloat16
FP8 = mybir.dt.float8e4
AF = mybir.ActivationFunctionType
ALU = mybir.AluOpType
DR = mybir.MatmulPerfMode.DoubleRow

NP_FP8 = ml_dtypes.float8_e4m3
NP_BF16 = ml_dtypes.bfloat16

# power-of-2 quantization scales (folded back out in on-chip epilogues)
S_X = 16.0
S_W1 = 1024.0
S_H = 32.0
S_WAB = 4096.0
SC_H = S_H / (S_X * S_W1)  # psum -> h units
SC_AT = 1.0 / (S_H * S_WAB)  # gating psum -> pre-activation units


def _build_tile_kernel(ctx: ExitStack, tc: tile.TileContext, t, npc, nblocks, zero_bias):
    nc = tc.nc

    singles = ctx.enter_context(tc.tile_pool(name="singles", bufs=1))
    xpool = ctx.enter_context(tc.tile_pool(name="xp", bufs=5))
    actp = ctx.enter_context(tc.tile_pool(name="actp", bufs=3))
    psum = ctx.enter_context(tc.tile_pool(name="psum", bufs=2, space=bass.MemorySpace.PSUM))

    # Block-0 x DMA first in program order: it is on the PE's critical path
    # (weights ride a separate HWDGE ring and overlap it).
    x_tiles0 = xpool.tile([128, 4, 2, NB], FP8, tag="x")
    nc.sync.dma_start(
        out=x_tiles0,
        in_=t["xt"][:, 0 : 8 * NB].rearrange("p (c i j) -> p c i j", i=2, j=NB),
    )

    # ---- persistent weights / biases in SBUF --------------------------------
    w1_sb = singles.tile([128, 4, 2, 2, 128], FP8, name="w1_sb")
    nc.scalar.dma_start(out=w1_sb, in_=t["w1q"].rearrange("p (c i m j) -> p c i m j", i=2, m=2, j=128))
    wa_sb = singles.tile([128, 2, 2, 128], FP8, name="wa_sb")
    nc.scalar.dma_start(out=wa_sb, in_=t["waq"].rearrange("p (i m j) -> p i m j", m=2, j=128))
    wb_sb = singles.tile([128, 2, 2, 128], FP8, name="wb_sb")
    nc.scalar.dma_start(out=wb_sb, in_=t["wbq"].rearrange("p (i m j) -> p i m j", m=2, j=128))
    ac_sb = singles.tile([128, 2, 1], BF16, name="ac_sb")
    nc.scalar.dma_start(out=ac_sb, in_=t["ach"].rearrange("p (k o) -> p k o", o=1))

    if not zero_bias:
        b1_sb = singles.tile([128, 2], F32, name="b1_sb")
        nc.scalar.dma_start(out=b1_sb, in_=t["b1s"].rearrange("(m p) -> p m", p=128))
        ba_sb = singles.tile([128, 2], F32, name="ba_sb")
        nc.scalar.dma_start(out=ba_sb, in_=t["bas"].rearrange("(m p) -> p m", p=128))
        bb_sb = singles.tile([128, 2], F32, name="bb_sb")
        nc.scalar.dma_start(out=bb_sb, in_=t["bbs"].rearrange("(m p) -> p m", p=128))

    s_parts = singles.tile([128, 2, nblocks], F32)
    z_parts = singles.tile([1, nblocks], F32)

    h_tiles = {}
    g_tiles = {}

    def h_phase(b):
        nb = min(NB, npc - b * NB)
        if b == 0:
            x_tile = x_tiles0
        else:
            x_tile = xpool.tile([128, 4, 2, NB], FP8, tag="x")
            nc.sync.dma_start(
                out=x_tile,
                in_=t["xt"][:, b * 8 * NB : (b + 1) * 8 * NB].rearrange("p (c i j) -> p c i j", i=2, j=NB),
            )

        # h^T = relu(W1^T x^T + b1), stored as fp8 (scaled by S_H).
        # Per-m psum banks with bufs=1: relu(m) drains while the other m's
        # matmuls run, so the next block's matmuls never wait.
        h_sb = actp.tile([128, 2, NB], FP8, tag="h", bufs=4)
        for m in range(2):
            ph = psum.tile([128, NB], F32, tag=f"ph{m}", bufs=1)
            if USE_DR:
                for c in range(4):
                    nc.tensor.matmul(ph[:, :nb], w1_sb[:, c, :, m, :], x_tile[:, c, :, :nb],
                                     perf_mode=DR, start=(c == 0), stop=(c == 3))
            else:
                for c in range(4):
                    for i in range(2):
                        nc.tensor.matmul(ph[:, :nb], w1_sb[:, c, i, m, :], x_tile[:, c, i, :nb],
                                         start=(c == 0 and i == 0), stop=(c == 3 and i == 1))
            if zero_bias:
                nc.vector.tensor_scalar(out=h_sb[:, m, :nb], in0=ph[:, :nb], scalar1=SC_H,
                                        scalar2=0.0, op0=ALU.mult, op1=ALU.max)
            else:
                nc.scalar.activation(out=h_sb[:, m, :nb], in_=ph[:, :nb], func=AF.Relu,
                                     bias=b1_sb[:, m : m + 1], scale=SC_H)
        h_tiles[b] = h_sb

    def gate_a(b):
        """a/t projections + tanh + g (on GpSimd)."""
        nb = min(NB, npc - b * NB)
        h_sb = h_tiles[b]

        # a = tanh(h Wa' + ba');  t = tanh(h Wb' + bb')  (0.5s folded host-side)
        pa = psum.tile([128, 2, NB], F32, tag="pa", bufs=1)
        pt = psum.tile([128, 2, NB], F32, tag="pt", bufs=1)
        for m in range(2):
            if USE_DR:
                nc.tensor.matmul(pa[:, m, :nb], wa_sb[:, :, m, :], h_sb[:, :, :nb], perf_mode=DR)
                nc.tensor.matmul(pt[:, m, :nb], wb_sb[:, :, m, :], h_sb[:, :, :nb], perf_mode=DR)
            else:
                for i in range(2):
                    nc.tensor.matmul(pa[:, m, :nb], wa_sb[:, i, m, :], h_sb[:, i, :nb],
                                     start=(i == 0), stop=(i == 1))
                for i in range(2):
                    nc.tensor.matmul(pt[:, m, :nb], wb_sb[:, i, m, :], h_sb[:, i, :nb],
                                     start=(i == 0), stop=(i == 1))
        a_sb = actp.tile([128, 2, NB], BF16, tag="a")
        t_sb = actp.tile([128, 2, NB], BF16, tag="t")
        if zero_bias:
            nc.scalar.activation(out=a_sb[:, :, :nb], in_=pa[:, :, :nb], func=AF.Tanh, scale=SC_AT)
            nc.scalar.activation(out=t_sb[:, :, :nb], in_=pt[:, :, :nb], func=AF.Tanh, scale=SC_AT)
        else:
            for m in range(2):
                nc.scalar.activation(out=a_sb[:, m, :nb], in_=pa[:, m, :nb], func=AF.Tanh,
                                     bias=ba_sb[:, m : m + 1], scale=SC_AT)
                nc.scalar.activation(out=t_sb[:, m, :nb], in_=pt[:, m, :nb], func=AF.Tanh,
                                     bias=bb_sb[:, m : m + 1], scale=SC_AT)

        # g = a * (1 + t)
        g_sb = actp.tile([128, 2, NB], BF16, tag="g")
        nc.vector.scalar_tensor_tensor(out=g_sb[:, :, :nb], in0=t_sb[:, :, :nb], scalar=1.0,
                                       in1=a_sb[:, :, :nb], op0=ALU.add, op1=ALU.mult)
        g_tiles[b] = g_sb

    def gate_b(b):
        """A projection, softmax weight, weighted pooling partials."""
        nb = min(NB, npc - b * NB)
        h_sb = h_tiles.pop(b)
        g_sb = g_tiles.pop(b)

        # A = g @ (0.5 ac) -> (1, nb);  w = exp(A); Z += sum(w)
        pA = psum.tile([1, NB], F32, tag="pA", bufs=2)
        for k in range(2):
            nc.tensor.matmul(pA[:, :nb], ac_sb[:, k, :], g_sb[:, k, :nb], start=(k == 0), stop=(k == 1))
        w_sb = actp.tile([1, NB], BF16, tag="w")
        nc.scalar.activation(out=w_sb[:, :nb], in_=pA[:, :nb], func=AF.Exp, scale=1.0,
                             accum_out=z_parts[:, b : b + 1])

        # broadcast w to all partitions (GpSimd), then S[:,m,b] += rowsum(h/S_H * w)
        wb_bc = actp.tile([128, NB], BF16, tag="wb")
        nc.gpsimd.partition_broadcast(wb_bc[:, :nb], w_sb[:, :nb])
        for m in range(2):
            wf = actp.tile([128, NB], BF16, tag="wf")
            nc.vector.scalar_tensor_tensor(out=wf[:, :nb], in0=h_sb[:, m, :nb], scalar=1.0 / S_H,
                                           in1=wb_bc[:, :nb], op0=ALU.mult, op1=ALU.mult,
                                           accum_out=s_parts[:, m, b : b + 1])

    # Software pipeline: gate_a runs one block late, gate_b two blocks late,
    # so no engine FIFO ever stalls on the cross-engine chain
    # (relu -> a/t MMs -> tanh -> g -> A MM -> exp -> bcast -> weighted sum)
    # and the PE stays continuously busy (HAM stays warm).
    for b in range(nblocks):
        h_phase(b)
        if b >= 1:
            gate_a(b - 1)
        if b >= 2:
            gate_b(b - 2)
    gate_a(nblocks - 1)
    gate_b(nblocks - 2)
    gate_b(nblocks - 1)

    nc.sync.dma_start(out=t["s_out"], in_=s_parts)
    nc.sync.dma_start(out=t["z_out"], in_=z_parts)


def build_program(npc: int = NPC, zero_bias: bool = True, enable_asserts: bool = False):
    nblocks = (npc + NB - 1) // NB
    nc = bacc.Bacc("TRN2", target_bir_lowering=False, debug=False, enable_asserts=enable_asserts)

    t = {}
    t["xt"] = nc.dram_tensor("xt", [128, nblocks * 8 * NB], FP8, kind="ExternalInput").ap()
    t["w1q"] = nc.dram_tensor("w1q", [128, 2048], FP8, kind="ExternalInput").ap()
    t["waq"] = nc.dram_tensor("waq", [128, 512], FP8, kind="ExternalInput").ap()
    t["wbq"] = nc.dram_tensor("wbq", [128, 512], FP8, kind="ExternalInput").ap()
    t["ach"] = nc.dram_tensor("ach", [128, 2], BF16, kind="ExternalInput").ap()
    if not zero_bias:
        for nm in ("b1s", "bas", "bbs"):
            t[nm] = nc.dram_tensor(nm, [D_HID], F32, kind="ExternalInput").ap()
    t["s_out"] = nc.dram_tensor("s_out", [128, 2, nblocks], F32, kind="ExternalOutput").ap()
    t["z_out"] = nc.dram_tensor("z_out", [1, nblocks], F32, kind="ExternalOutput").ap()

    with tile.TileContext(nc) as tc, ExitStack() as ctx:
        _build_tile_kernel(ctx, tc, t, npc, nblocks, zero_bias)
    nc.compile()
    return nc


def _q8(a: np.ndarray, scale: float) -> np.ndarray:
    return np.ascontiguousarray((np.asarray(a, np.float32) * scale).astype(NP_FP8))


def make_weight_map(inputs, zero_bias=None):
    W1 = np.asarray(inputs["wsi_w"], np.float64)
    Wv = np.asarray(inputs["wv_w"], np.float64)
    Wa = np.asarray(inputs["aa_w"], np.float64)
    Wb = np.asarray(inputs["ab_w"], np.float64)
    ac = np.asarray(inputs["ac_w"], np.float64)
    bv = np.asarray(inputs["wv_b"], np.float64)
    b1 = np.asarray(inputs["wsi_b"], np.float64)
    ba = np.asarray(bv @ Wa + np.asarray(inputs["aa_b"], np.float64))
    bb = np.asarray(0.5 * (bv @ Wb + np.asarray(inputs["ab_b"], np.float64)))

    # composed gating weights (f folded away); 0.5 of the tanh-sigmoid in Wb'
    Wa_c = Wv @ Wa
    Wb_c = 0.5 * (Wv @ Wb)

    # device layouts
    w1q = _q8(W1, S_W1).reshape(4, 2, 128, 2, 128).transpose(2, 0, 1, 3, 4).reshape(128, 2048)
    waq = _q8(Wa_c, S_WAB).reshape(2, 128, 2, 128).transpose(1, 0, 2, 3).reshape(128, 512)
    wbq = _q8(Wb_c, S_WAB).reshape(2, 128, 2, 128).transpose(1, 0, 2, 3).reshape(128, 512)
    ach = np.ascontiguousarray(
        (0.5 * ac).astype(NP_BF16).reshape(2, 128, 1).transpose(1, 0, 2).reshape(128, 2)
    )
    m = {"w1q": np.ascontiguousarray(w1q), "waq": np.ascontiguousarray(waq),
         "wbq": np.ascontiguousarray(wbq), "ach": ach}
    zb = not (np.any(b1) or np.any(ba) or np.any(bb))
    if not zb:
        m["b1s"] = (np.asarray(b1, np.float32) * S_H).astype(np.float32)
        m["bas"] = np.asarray(ba, np.float32)
        m["bbs"] = np.asarray(bb, np.float32)
    m["_zero_bias"] = zb
    return m


def make_in_maps(x_path, weights, npc: int = NPC, n_cores: int = N_CORES):
    x = np.asarray(x_path[0], np.float32)  # (N, 1024)
    nblocks = (npc + NB - 1) // NB
    npad = nblocks * NB
    w = {k: v for k, v in weights.items() if not k.startswith("_")}
    in_maps = []
    for c in range(n_cores):
        xc = np.zeros((npad, D_IN), np.float32)
        xc[:npc] = x[c * npc : (c + 1) * npc]
        xq = (xc * S_X).astype(NP_FP8)
        # [inst, feat] -> [p, (b c i j)] with feat = c*256 + i*128 + p
        packed = np.ascontiguousarray(
            xq.reshape(nblocks, NB, 4, 2, 128).transpose(4, 0, 2, 3, 1).reshape(128, nblocks * 8 * NB)
        )
        in_maps.append({"xt": packed, **w})
    return in_maps


def finalize(results, c1_w, c1_b, c2_w, c2_b, wv_w, wv_b):
    """Host-side reduction of per-core partials, Wv application + classifier."""
    S = np.zeros((128, 2), np.float64)
    Z = 0.0
    for r in results:
        S += np.asarray(r["s_out"], np.float64).sum(axis=-1)
        Z += float(np.asarray(r["z_out"], np.float64).sum())
    s_vec = S.T.reshape(256)  # feature = m*128 + p
    pooled = (s_vec / Z) @ np.asarray(wv_w, np.float64) + np.asarray(wv_b, np.float64)
    risk = (
        np.maximum(pooled @ np.asarray(c1_w, np.float64) + np.asarray(c1_b, np.float64), 0.0)
        @ np.asarray(c2_w, np.float64)
        + np.asarray(c2_b, np.float64)
    )
    return risk[None, :].astype(np.float32)


_CACHED = {}


def kernel(**inputs) -> np.ndarray:
    weights = make_weight_map(inputs)
    zb = weights["_zero_bias"]
    if zb not in _CACHED:
        _CACHED[zb] = build_program(zero_bias=zb)
    nc = _CACHED[zb]

    in_maps = make_in_maps(np.asarray(inputs["x_path"]), weights)
    res = run_bass_kernel_spmd(nc, in_maps, list(range(N_CORES)))
    return finalize(
        res.results,
        inputs["c1_w"], inputs["c1_b"], inputs["c2_w"], inputs["c2_b"],
        inputs["wv_w"], inputs["wv_b"],
    )


# revision 6
# speedup vs baseline: 1.8368x; 1.0695x over previous
"""Trainium2 Bass kernel for the MCAT gated-attention MIL pooling model.

Math (reference after dead-code elimination + algebraic folding):
  The per-instance cross-attention softmax is over a length-1 axis -> attn_w == 1,
  so fused = v = h @ Wv + bv with h = relu(x_path @ W1 + b1).  The x_cell / wq /
  wk branch is dead.

  Key folding: f( = v) is LINEAR in h, so
    - gating:  f @ Wa = h @ (Wv Wa) + (bv Wa)   -> composed weights on the host
    - pooling: sum_n w_n f_n = (sum_n w_n h_n) @ Wv + bv * sum_n w_n
  The device therefore never materializes f at all:
      h   = relu(x @ W1 + b1)                  (N, 256)
      a   = tanh(h @ Wa' + ba')                Wa' = Wv Wa,      ba' = bv Wa + ba
      t   = tanh(h @ Wb' + bb')                Wb' = 0.5 Wv Wb,  bb' = 0.5 (bv Wb + bb)
      A   = (a * (1 + t)) @ (0.5 ac)           (sigmoid(y) = 0.5 (1 + tanh(y/2)))
      w   = exp(A)          (the ac_b bias cancels in S/Z and is dropped)
      S  += w_n * h_n ;  Z += w_n              per-core partial sums
  Host: pooled = (S/Z) @ Wv + bv ; risk = relu(pooled @ c1 + b) @ c2 + b2  (fp64).

Precision: rel-err budget is 2e-2; measured host study gives 2.3e-3 with x/W1/h
and the gating weights in fp8(e4m3, power-of-2 scaled) and everything else bf16.
fp8 enables DoubleRow matmuls (2 contraction rows per PE cell) for the dominant
x@W1 (8 MMs/block instead of 16) and the gating projections (2 each instead of 4).
Scales are powers of two folded into ACT/DVE epilogues (exact).

Sharding: rows split across 8 cores (6250 each); cores return per-block partial
sums S (128,2,NB) and Z (1,NB); host reduces in fp64 + tiny classifier.
"""

import sys
from contextlib import ExitStack

import numpy as np

try:
    import concourse  # noqa: F401
except ImportError:  # pragma: no cover - fresh grading env
    sys.path.insert(0, "/opt/trn_rl_repo")

import ml_dtypes

import concourse.bass as bass
import concourse.tile as tile
from concourse import bacc, mybir
from concourse.bass_utils import run_bass_kernel_spmd

N_CORES = 8
N = 50000
NPC = N // N_CORES  # 6250 rows per core
D_IN = 1024
D_HID = 256
NB = 512  # instances per block (one PSUM bank of fp32)
USE_DR = True  # DoubleRow fp8 matmuls (2 contraction rows/cell)

F32 = mybir.dt.float32
BF16 = mybir.dt.bfloat16
FP8 = mybir.dt.float8e4
AF = mybir.ActivationFunctionType
ALU = mybir.AluOpType
DR = mybir.MatmulPerfMode.DoubleRow

NP_FP8 = ml_dtypes.float8_e4m3
NP_BF16 = ml_dtypes.bfloat16

# power-of-2 quantization scales (folded back out in on-chip epilogues)
S_X = 16.0
S_W1 = 1024.0
S_H = 32.0
S_WAB = 4096.0
SC_H = S_H / (S_X * S_W1)  # psum -> h units
SC_AT = 1.0 / (S_H * S_WAB)  # gating psum -> pre-activation units


def _build_tile_kernel(ctx: ExitStack, tc: tile.TileContext, t, npc, nblocks, zero_bias):
    nc = tc.nc

    singles = ctx.enter_context(tc.tile_pool(name="singles", bufs=1))
    xpool = ctx.enter_context(tc.tile_pool(name="xp", bufs=5))
    actp = ctx.enter_context(tc.tile_pool(name="actp", bufs=3))
    psum = ctx.enter_context(tc.tile_pool(name="psum", bufs=2, space=bass.MemorySpace.PSUM))

    # Block-0 x DMA first in program order: it is on the PE's critical path
    # (weights ride a separate HWDGE ring and overlap it).
    x_tiles0 = xpool.tile([128, 4, 2, NB], FP8, tag="x")
    nc.sync.dma_start(
        out=x_tiles0,
        in_=t["xt"][:, 0 : 8 * NB].rearrange("p (c i j) -> p c i j", i=2, j=NB),
    )

    # ---- persistent weights / biases in SBUF --------------------------------
    w1_sb = singles.tile([128, 4, 2, 2, 128], FP8, name="w1_sb")
    nc.scalar.dma_start(out=w1_sb, in_=t["w1q"].rearrange("p (c i m j) -> p c i m j", i=2, m=2, j=128))
    wa_sb = singles.tile([128, 2, 2, 128], FP8, name="wa_sb")
    nc.scalar.dma_start(out=wa_sb, in_=t["waq"].rearrange("p (i m j) -> p i m j", m=2, j=128))
    wb_sb = singles.tile([128, 2, 2, 128], FP8, name="wb_sb")
    nc.scalar.dma_start(out=wb_sb, in_=t["wbq"].rearrange("p (i m j) -> p i m j", m=2, j=128))
    ac_sb = singles.tile([128, 2, 1], BF16, name="ac_sb")
    nc.scalar.dma_start(out=ac_sb, in_=t["ach"].rearrange("p (k o) -> p k o", o=1))

    if not zero_bias:
        b1_sb = singles.tile([128, 2], F32, name="b1_sb")
        nc.scalar.dma_start(out=b1_sb, in_=t["b1s"].rearrange("(m p) -> p m", p=128))
        ba_sb = singles.tile([128, 2], F32, name="ba_sb")
        nc.scalar.dma_start(out=ba_sb, in_=t["bas"].rearrange("(m p) -> p m", p=128))
        bb_sb = singles.tile([128, 2], F32, name="bb_sb")
        nc.scalar.dma_start(out=bb_sb, in_=t["bbs"].rearrange("(m p) -> p m", p=128))

    s_parts = singles.tile([128, 2, nblocks], F32)
    z_parts = singles.tile([1, nblocks], F32)

    h_tiles = {}
    g_tiles = {}

    def h_phase(b):
        nb = min(NB, npc - b * NB)
        if b == 0:
            x_tile = x_tiles0
        else:
            x_tile = xpool.tile([128, 4, 2, NB], FP8, tag="x")
            nc.sync.dma_start(
                out=x_tile,
                in_=t["xt"][:, b * 8 * NB : (b + 1) * 8 * NB].rearrange("p (c i j) -> p c i j", i=2, j=NB),
            )

        # h^T = relu(W1^T x^T + b1), stored as fp8 (scaled by S_H).
        # Per-m psum banks with bufs=1: relu(m) drains while the other m's
        # matmuls run, so the next block's matmuls never wait.
        h_sb = actp.tile([128, 2, NB], FP8, tag="h", bufs=4)
        for m in range(2):
            ph = psum.tile([128, NB], F32, tag=f"ph{m}", bufs=1)
            if USE_DR:
                for c in range(4):
                    nc.tensor.matmul(ph[:, :nb], w1_sb[:, c, :, m, :], x_tile[:, c, :, :nb],
                                     perf_mode=DR, start=(c == 0), stop=(c == 3))
            else:
                for c in range(4):
                    for i in range(2):
                        nc.tensor.matmul(ph[:, :nb], w1_sb[:, c, i, m, :], x_tile[:, c, i, :nb],
                                         start=(c == 0 and i == 0), stop=(c == 3 and i == 1))
            if zero_bias:
                nc.vector.tensor_scalar(out=h_sb[:, m, :nb], in0=ph[:, :nb], scalar1=SC_H,
                                        scalar2=0.0, op0=ALU.mult, op1=ALU.max)
            else:
                nc.scalar.activation(out=h_sb[:, m, :nb], in_=ph[:, :nb], func=AF.Relu,
                                     bias=b1_sb[:, m : m + 1], scale=SC_H)
        h_tiles[b] = h_sb

    def gate_a(b):
        """a/t projections + tanh + u = a*t."""
        nb = min(NB, npc - b * NB)
        h_sb = h_tiles[b]

        # a = tanh(h Wa' + ba');  t = tanh(h Wb' + bb')  (0.5s folded host-side)
        # One 4-bank psum tile -> a single merged tanh op over [128, 4, nb].
        pat = psum.tile([128, 4, NB], F32, tag="pat", bufs=1)
        at_sb = actp.tile([128, 4, NB], BF16, tag="at")
        for m in range(2):
            if USE_DR:
                nc.tensor.matmul(pat[:, 0 + m, :nb], wa_sb[:, :, m, :], h_sb[:, :, :nb], perf_mode=DR)
                nc.tensor.matmul(pat[:, 2 + m, :nb], wb_sb[:, :, m, :], h_sb[:, :, :nb], perf_mode=DR)
            else:
                for i in range(2):
                    nc.tensor.matmul(pat[:, 0 + m, :nb], wa_sb[:, i, m, :], h_sb[:, i, :nb],
                                     start=(i == 0), stop=(i == 1))
                for i in range(2):
                    nc.tensor.matmul(pat[:, 2 + m, :nb], wb_sb[:, i, m, :], h_sb[:, i, :nb],
                                     start=(i == 0), stop=(i == 1))
        if zero_bias:
            nc.scalar.activation(out=at_sb[:, :, :nb], in_=pat[:, :, :nb], func=AF.Tanh, scale=SC_AT)
        else:
            for m in range(2):
                nc.scalar.activation(out=at_sb[:, 0 + m, :nb], in_=pat[:, 0 + m, :nb], func=AF.Tanh,
                                     bias=ba_sb[:, m : m + 1], scale=SC_AT)
                nc.scalar.activation(out=at_sb[:, 2 + m, :nb], in_=pat[:, 2 + m, :nb], func=AF.Tanh,
                                     bias=bb_sb[:, m : m + 1], scale=SC_AT)

        # u = a * t  (a*(1+t) = a + a*t is folded into two A-projections)
        u_sb = actp.tile([128, 2, NB], BF16, tag="u")
        nc.vector.tensor_tensor(out=u_sb[:, :, :nb], in0=at_sb[:, 0:2, :nb],
                                in1=at_sb[:, 2:4, :nb], op=ALU.mult)
        g_tiles[b] = (at_sb, u_sb)

    def gate_b(b):
        """A projection, softmax weight, weighted pooling partials."""
        nb = min(NB, npc - b * NB)
        h_sb = h_tiles.pop(b)
        at_sb, u_sb = g_tiles.pop(b)

        # A = (a + a*t) @ (0.5 ac) -> (1, nb);  w = exp(A); Z += sum(w)
        pA = psum.tile([1, NB], F32, tag="pA", bufs=1)
        for k in range(2):
            nc.tensor.matmul(pA[:, :nb], ac_sb[:, k, :], at_sb[:, k, :nb], start=(k == 0), stop=False)
        for k in range(2):
            nc.tensor.matmul(pA[:, :nb], ac_sb[:, k, :], u_sb[:, k, :nb], start=False, stop=(k == 1))
        w_sb = actp.tile([1, NB], BF16, tag="w")
        nc.scalar.activation(out=w_sb[:, :nb], in_=pA[:, :nb], func=AF.Exp, scale=1.0,
                             accum_out=z_parts[:, b : b + 1])

        # broadcast w to all partitions (GpSimd), then S[:,m,b] += rowsum(h/S_H * w)
        wb_bc = actp.tile([128, NB], BF16, tag="wb")
        nc.gpsimd.partition_broadcast(wb_bc[:, :nb], w_sb[:, :nb])
        for m in range(2):
            wf = actp.tile([128, NB], BF16, tag="wf")
            nc.vector.scalar_tensor_tensor(out=wf[:, :nb], in0=h_sb[:, m, :nb], scalar=1.0 / S_H,
                                           in1=wb_bc[:, :nb], op0=ALU.mult, op1=ALU.mult,
                                           accum_out=s_parts[:, m, b : b + 1])

    # Software pipeline: gate_a runs one block late, gate_b two blocks late,
    # so no engine FIFO ever stalls on the cross-engine chain
    # (relu -> a/t MMs -> tanh -> g -> A MM -> exp -> bcast -> weighted sum)
    # and the PE stays continuously busy (HAM stays warm).
    for b in range(nblocks):
        h_phase(b)
        if b >= 1:
            gate_a(b - 1)
        if b >= 2:
            gate_b(b - 2)
    gate_a(nblocks - 1)
    gate_b(nblocks - 2)
    gate_b(nblocks - 1)

    nc.sync.dma_start(out=t["s_out"], in_=s_parts)
    nc.sync.dma_start(out=t["z_out"], in_=z_parts)


def build_program(npc: int = NPC, zero_bias: bool = True, enable_asserts: bool = False):
    nblocks = (npc + NB - 1) // NB
    nc = bacc.Bacc("TRN2", target_bir_lowering=False, debug=False, enable_asserts=enable_asserts)

    t = {}
    t["xt"] = nc.dram_tensor("xt", [128, nblocks * 8 * NB], FP8, kind="ExternalInput").ap()
    t["w1q"] = nc.dram_tensor("w1q", [128, 2048], FP8, kind="ExternalInput").ap()
    t["waq"] = nc.dram_tensor("waq", [128, 512], FP8, kind="ExternalInput").ap()
    t["wbq"] = nc.dram_tensor("wbq", [128, 512], FP8, kind="ExternalInput").ap()
    t["ach"] = nc.dram_tensor("ach", [128, 2], BF16, kind="ExternalInput").ap()
    if not zero_bias:
        for nm in ("b1s", "bas", "bbs"):
            t[nm] = nc.dram_tensor(nm, [D_HID], F32, kind="ExternalInput").ap()
    t["s_out"] = nc.dram_tensor("s_out", [128, 2, nblocks], F32, kind="ExternalOutput").ap()
    t["z_out"] = nc.dram_tensor("z_out", [1, nblocks], F32, kind="ExternalOutput").ap()

    with tile.TileContext(nc) as tc, ExitStack() as ctx:
        _build_tile_kernel(ctx, tc, t, npc, nblocks, zero_bias)
    nc.compile()
    return nc


def _q8(a: np.ndarray, scale: float) -> np.ndarray:
    return np.ascontiguousarray((np.asarray(a, np.float32) * scale).astype(NP_FP8))


def make_weight_map(inputs, zero_bias=None):
    W1 = np.asarray(inputs["wsi_w"], np.float64)
    Wv = np.asarray(inputs["wv_w"], np.float64)
    Wa = np.asarray(inputs["aa_w"], np.float64)
    Wb = np.asarray(inputs["ab_w"], np.float64)
    ac = np.asarray(inputs["ac_w"], np.float64)
    bv = np.asarray(inputs["wv_b"], np.float64)
    b1 = np.asarray(inputs["wsi_b"], np.float64)
    ba = np.asarray(bv @ Wa + np.asarray(inputs["aa_b"], np.float64))
    bb = np.asarray(0.5 * (bv @ Wb + np.asarray(inputs["ab_b"], np.float64)))

    # composed gating weights (f folded away); 0.5 of the tanh-sigmoid in Wb'
    Wa_c = Wv @ Wa
    Wb_c = 0.5 * (Wv @ Wb)

    # device layouts
    w1q = _q8(W1, S_W1).reshape(4, 2, 128, 2, 128).transpose(2, 0, 1, 3, 4).reshape(128, 2048)
    waq = _q8(Wa_c, S_WAB).reshape(2, 128, 2, 128).transpose(1, 0, 2, 3).reshape(128, 512)
    wbq = _q8(Wb_c, S_WAB).reshape(2, 128, 2, 128).transpose(1, 0, 2, 3).reshape(128, 512)
    ach = np.ascontiguousarray(
        (0.5 * ac).astype(NP_BF16).reshape(2, 128, 1).transpose(1, 0, 2).reshape(128, 2)
    )
    m = {"w1q": np.ascontiguousarray(w1q), "waq": np.ascontiguousarray(waq),
         "wbq": np.ascontiguousarray(wbq), "ach": ach}
    zb = not (np.any(b1) or np.any(ba) or np.any(bb))
    if not zb:
        m["b1s"] = (np.asarray(b1, np.float32) * S_H).astype(np.float32)
        m["bas"] = np.asarray(ba, np.float32)
        m["bbs"] = np.asarray(bb, np.float32)
    m["_zero_bias"] = zb
    return m


def make_in_maps(x_path, weights, npc: int = NPC, n_cores: int = N_CORES):
    x = np.asarray(x_path[0], np.float32)  # (N, 1024)
    nblocks = (npc + NB - 1) // NB
    npad = nblocks * NB
    w = {k: v for k, v in weights.items() if not k.startswith("_")}
    in_maps = []
    for c in range(n_cores):
        xc = np.zeros((npad, D_IN), np.float32)
        xc[:npc] = x[c * npc : (c + 1) * npc]
        xq = (xc * S_X).astype(NP_FP8)
        # [inst, feat] -> [p, (b c i j)] with feat = c*256 + i*128 + p
        packed = np.ascontiguousarray(
            xq.reshape(nblocks, NB, 4, 2, 128).transpose(4, 0, 2, 3, 1).reshape(128, nblocks * 8 * NB)
        )
        in_maps.append({"xt": packed, **w})
    return in_maps


def finalize(results, c1_w, c1_b, c2_w, c2_b, wv_w, wv_b):
    """Host-side reduction of per-core partials, Wv application + classifier."""
    S = np.zeros((128, 2), np.float64)
    Z = 0.0
    for r in results:
        S += np.asarray(r["s_out"], np.float64).sum(axis=-1)
        Z += float(np.asarray(r["z_out"], np.float64).sum())
    s_vec = S.T.reshape(256)  # feature = m*128 + p
    pooled = (s_vec / Z) @ np.asarray(wv_w, np.float64) + np.asarray(wv_b, np.float64)
    risk = (
        np.maximum(pooled @ np.asarray(c1_w, np.float64) + np.asarray(c1_b, np.float64), 0.0)
        @ np.asarray(c2_w, np.float64)
        + np.asarray(c2_b, np.float64)
    )
    return risk[None, :].astype(np.float32)


_CACHED = {}


def kernel(**inputs) -> np.ndarray:
    weights = make_weight_map(inputs)
    zb = weights["_zero_bias"]
    if zb not in _CACHED:
        _CACHED[zb] = build_program(zero_bias=zb)
    nc = _CACHED[zb]

    in_maps = make_in_maps(np.asarray(inputs["x_path"]), weights)
    res = run_bass_kernel_spmd(nc, in_maps, list(range(N_CORES)))
    return finalize(
        res.results,
        inputs["c1_w"], inputs["c1_b"], inputs["c2_w"], inputs["c2_b"],
        inputs["wv_w"], inputs["wv_b"],
    )
